# revision 6
# baseline (speedup 1.0000x reference)
"""Segment-prefix max kernel for Trainium2 (8 NeuronCores, SPMD).

Problem: x [1048576, 128] f32, 2048 uniform segments of 512 rows each;
out[i, :] = max over the first (512 - window_size + 1) rows of segment i.

Strategy (memory-bound, ~512 MiB streamed from HBM):
  - Shard segments across 8 cores: core c gets rows [c*131072, (c+1)*131072)
    and produces out rows [c*256, (c+1)*256). No cross-core communication.
  - Per core, 2 MiB tiles of 8 segments. Partition 16*l + i holds rows
    [32*i, 32*i+32) of segment l — one 16 KiB contiguous DMA run per
    partition (vs 2 KiB in the naive layout), which lifts the per-core DMA
    ceiling from ~320 GB/s to ~550+ GB/s (per-packet overhead dominates
    small packets).
  - Tail partitions (i == 15) initially hold rows 480..511, which include
    the invalid window tail; a second strided DMA overwrites them with the
    last 32 VALID rows [count-32, count) (duplicate reads are harmless for
    max). Costs +6% HBM traffic, avoids any ragged compute.
  - The 32 -> 1 fold along the free axis runs on DVE as a binary tree; the
    first level reads f32 and writes bf16, middle levels run in bf16 at 2x
    DVE throughput, the last level emits f32 (rel tolerance 2e-2 >> bf16's
    ~4e-3 rounding).
  - Cross-partition max (each segment = 16 consecutive partitions) goes
    through a PE transpose (identity matmul into PSUM) and one DVE
    reduce_max along the free axis, yielding 8 output columns per tile.
  - Columns accumulate in an SBUF [128, n_seg] f32 buffer, PE-transposed
    back to row-major [n_seg, 128] chunks and DMA'd out.
"""

import numpy as np

import concourse.bacc as bacc
import concourse.bass as bass
import concourse.tile as tile
from concourse import mybir
from concourse.bass_utils import run_bass_kernel_spmd
from concourse.masks import make_identity

N_CORES = 8
SEG_LEN = 512
D = 128
J = 32  # rows per partition per tile (16 KiB contiguous DMA run)
RUNS = SEG_LEN // J  # 16 partitions per segment
SEGS_PER_TILE = 128 // RUNS  # 8 segments * 4096 rows * 512 B = 2 MiB tiles

_PROGRAM_CACHE: dict = {}


def _build_program(n_seg_core: int, count: int) -> bacc.Bacc:
    """Bass program for one core: n_seg_core segments, max over first
    `count` rows of each. Requires SEG_LEN - J < count <= SEG_LEN."""
    assert SEG_LEN - J < count <= SEG_LEN
    rows = n_seg_core * SEG_LEN
    n_tiles = n_seg_core // SEGS_PER_TILE
    f32 = mybir.dt.float32
    bf16 = mybir.dt.bfloat16

    nc = bacc.Bacc("TRN2", target_bir_lowering=False, debug=False)
    x_in = nc.dram_tensor("x", [rows, D], f32, kind="ExternalInput")
    out_t = nc.dram_tensor("out", [n_seg_core, D], f32, kind="ExternalOutput")

    # partition-major view: partition p of tile t holds rows of run p
    x_flat = x_in.rearrange("(p j) d -> p j d", j=J)
    # row view per segment for the tail reload
    x_rows = x_in.rearrange("(g q) d -> g q d", q=SEG_LEN)

    with tile.TileContext(nc) as tc:
        with (
            tc.tile_pool(name="io", bufs=8) as io_pool,
            tc.tile_pool(name="work", bufs=4) as work_pool,
            tc.tile_pool(name="psum", bufs=4, space="PSUM") as psum_pool,
            tc.tile_pool(name="consts", bufs=1) as consts,
        ):
            ident_f = consts.tile([128, 128], f32)
            make_identity(nc, ident_f)
            outbuf = consts.tile([128, n_seg_core], f32)

            for t in range(n_tiles):
                tl = io_pool.tile([128, J, D], f32, tag="tl")
                hw = nc.sync if t % 2 == 0 else nc.scalar
                g0 = t * SEGS_PER_TILE
                hw.dma_start(out=tl, in_=x_flat[t * 128 : (t + 1) * 128])
                if count < SEG_LEN:
                    # overwrite tail partitions with the last 32 valid rows
                    hw.dma_start(
                        out=tl[RUNS - 1 :: RUNS],
                        in_=x_rows[g0 : g0 + SEGS_PER_TILE, count - J : count],
                    )

                # fold 32 -> 1 along j: f32 -> bf16, bf16 tree, bf16 -> f32
                w = work_pool.tile([128, J // 2, D], bf16, tag="w")
                nc.vector.tensor_max(out=w, in0=tl[:, : J // 2], in1=tl[:, J // 2 :])
                k = J // 2
                while k > 2:
                    k //= 2
                    nc.vector.tensor_max(
                        out=w[:, :k], in0=w[:, :k], in1=w[:, k : 2 * k]
                    )
                wf = work_pool.tile([128, D], f32, tag="wf")
                nc.vector.tensor_max(out=wf, in0=w[:, 0], in1=w[:, 1])

                # cross-partition max: transpose, reduce 16-column groups
                pt = psum_pool.tile([128, SEGS_PER_TILE, RUNS], f32, tag="pt")
                nc.tensor.transpose(pt.rearrange("p a b -> p (a b)"), wf, ident_f)
                nc.vector.reduce_max(
                    out=outbuf[:, g0 : g0 + SEGS_PER_TILE],
                    in_=pt,
                    axis=mybir.AxisListType.X,
                )

            # outbuf is [128 d, n_seg_core]; transpose back to [seg, d]
            for c in range(n_seg_core // 128):
                pt = psum_pool.tile([128, 128], f32, tag="ot_ps")
                nc.tensor.transpose(pt, outbuf[:, c * 128 : (c + 1) * 128], ident_f)
                ot = io_pool.tile([128, 128], f32, tag="ot")
                nc.scalar.copy(ot, pt)
                nc.sync.dma_start(out=out_t[c * 128 : (c + 1) * 128, :], in_=ot)
    nc.compile()
    return nc


def _numpy_fallback(x: np.ndarray, sizes: np.ndarray, w: int) -> np.ndarray:
    ends = np.cumsum(sizes)
    starts = ends - sizes
    out = np.full((sizes.shape[0], x.shape[1]), -np.inf, dtype=np.float32)
    for i in range(sizes.shape[0]):
        c = int(sizes[i]) - w + 1
        if c > 0:
            out[i] = x[int(starts[i]) : int(starts[i]) + c].max(axis=0)
    return out


def kernel(x, sizes, window_size) -> np.ndarray:
    x = np.ascontiguousarray(np.asarray(x, dtype=np.float32))
    sizes = np.asarray(sizes)
    w = int(np.asarray(window_size))
    n_seg = sizes.shape[0]
    count = SEG_LEN - w + 1

    uniform = (
        x.ndim == 2
        and x.shape[1] == D
        and bool((sizes == SEG_LEN).all())
        and x.shape[0] == n_seg * SEG_LEN
        and n_seg % (N_CORES * SEGS_PER_TILE) == 0
        and (n_seg // N_CORES) % 128 == 0
        and SEG_LEN - J < count <= SEG_LEN
    )
    if not uniform:
        return _numpy_fallback(x, sizes, w)

    n_seg_core = n_seg // N_CORES
    key = (n_seg_core, count)
    if key not in _PROGRAM_CACHE:
        _PROGRAM_CACHE[key] = _build_program(n_seg_core, count)
    nc = _PROGRAM_CACHE[key]

    shards = np.split(x, N_CORES, axis=0)
    in_maps = [{"x": s} for s in shards]
    res = run_bass_kernel_spmd(nc, in_maps, core_ids=list(range(N_CORES)))
    return np.concatenate([r["out"] for r in res.results], axis=0)
